# revision 1
# baseline (speedup 1.0000x reference)
"""Trainium2 Bass kernel for CropProposals (adaptive max-pool 2x2x2 over
data-dependent crops of a [4,128,24,24,24] feature map).

Sharding: core k = 2*b + h handles batch b with a load-balanced half of the
64 proposals (full 128-channel dim on SBUF partitions).  All crop bounds are
computed on the host from `corners` (tiny int math) and baked into the Bass
program as static access patterns; per-core differences live in 8
partition-id branches of one SPMD program.  Each octant pair (oz in {0,1})
of a proposal is one VectorE tensor_reduce over a strided 5-D access
pattern [C][oz][d][h][w] reducing d/h/w.
"""

import numpy as np

_B, _C, _D, _H, _W = 4, 128, 24, 24, 24
_P = 64
_NCORES = 8
_PPC = _P // 2          # proposals per core
_VOL = _D * _H * _W     # 13824
_SD, _SH, _SW = _H * _W, _W, 1   # element strides of [D,H,W] volume

_cache = {}


def _box_params(corners, scale):
    """Host-side replica of the reference bound math.

    Returns s, l, dlt arrays of shape [B, P, 3] (axis order D,H,W):
      region(o) along axis a = [ s + o*dlt , s + o*dlt + l )
    """
    c = np.asarray(corners).astype(np.int64)
    p1 = np.clip(c[:, :, 0, :] // scale, 0, 21)
    p2r = c[:, :, 1, :] // scale
    p2 = np.where(p2r - p1 >= 2, p2r, p1 + 2)
    sizes = np.array([_D, _H, _W], dtype=np.int64)
    e = np.minimum(p2, sizes)
    n = e - p1                 # crop length per axis, >= 2
    l = (n + 1) // 2           # region length (same for both regions)
    dlt = n // 2               # region-1 start offset from region-0 start
    return p1, l, dlt


def _assign_proposals(s, l, dlt):
    """Balance proposals between the two cores of each batch by estimated
    VectorE cycles (8*vol + fixed per-proposal instruction overhead)."""
    assign = []   # per batch: (idx_core0, idx_core1)
    for b in range(_B):
        vol = l[b].prod(axis=-1)
        cost = 8 * vol + 290
        order = np.argsort(-cost)
        loads = [0, 0]
        sets = [[], []]
        for p in order:
            k = 0 if (loads[0] <= loads[1] and len(sets[0]) < _PPC) or len(sets[1]) >= _PPC else 1
            sets[k].append(int(p))
            loads[k] += int(cost[p])
        assign.append((sets[0], sets[1]))
    return assign


def _build_program(s, l, dlt, assign):
    import concourse.bacc as bacc
    import concourse.mybir as mybir
    from concourse.tile import TileContext
    from concourse.ap import AP

    nc = bacc.Bacc("TRN2", target_bir_lowering=False, debug=False,
                   num_devices=_NCORES)
    x_in = nc.dram_tensor("fm", [_C, _VOL], mybir.dt.float32,
                          kind="ExternalInput")
    y_out = nc.dram_tensor("out", [_C, _PPC * 8], mybir.dt.float32,
                           kind="ExternalOutput")

    n_chunks = 6
    dpc = _D // n_chunks      # D planes per chunk

    with TileContext(nc) as tc:
        with tc.tile_pool(name="pool", bufs=1) as pool:
            xt = pool.tile([_C, _VOL], mybir.dt.float32)
            yt = pool.tile([_C, _PPC * 8], mybir.dt.float32)
            for ci in range(n_chunks):
                sl = slice(ci * dpc * _SD, (ci + 1) * dpc * _SD)
                nc.sync.dma_start(out=xt[:, sl], in_=x_in[:, sl])
            # restrict the partition-id register (and therefore the If
            # branches) to the Vector engine: the other 4 engines then skip
            # the whole branch cascade instead of walking 8 blocks of
            # event-semaphore choreography (~13us on the measured trace)
            pid = nc.partition_id(engines=(mybir.EngineType.DVE,))
            base = xt[:]
            part_dim = list(base.ap[0])
            for k in range(_NCORES):
                b, h = k // 2, k % 2
                plist = assign[b][h]
                # issue proposals in order of max D index touched so early
                # DMA chunks unblock early reduces
                plist = sorted(plist, key=lambda p: s[b, p, 0] + dlt[b, p, 0] + l[b, p, 0])
                # false-path fallthrough: the 8 condition checks pack into one
                # IRAM block and each core takes a single far jump into its
                # own body (instead of hopping over every other body)
                with tc.If(pid == k, preferred_fallthrough_block=False):
                    for j, p in enumerate(plist):
                        sx, sy, sz = (int(v) for v in s[b, p])
                        lx, ly, lz = (int(v) for v in l[b, p])
                        dx, dy, dz = (int(v) for v in dlt[b, p])
                        for ox in range(2):
                            for oy in range(2):
                                off = ((sx + ox * dx) * _SD
                                       + (sy + oy * dy) * _SH + sz)
                                ap = AP(base.tensor, base.offset + off,
                                        [part_dim, [dz, 2], [_SD, lx],
                                         [_SH, ly], [1, lz]])
                                col = j * 8 + ox * 4 + oy * 2
                                nc.vector.tensor_reduce(
                                    out=yt[:, col:col + 2], in_=ap,
                                    axis=mybir.AxisListType.XYZ,
                                    op=mybir.AluOpType.max)
            nc.sync.dma_start(out=y_out[:], in_=yt[:])
    nc.compile()
    return nc


_CHUNK_BOUNDS = [0, 3, 6, 9, 12, 15, 18, 21, 24]
_T0, _RATE, _RCPT, _VSTART = 8300.0, 760.0, 2200.0, 15000.0


def _chunk_req(smax, bounds):
    return next(i for i in range(len(bounds) - 1) if bounds[i + 1] >= smax)


def _sim_finish(items, bounds):
    """items: list of (chunk_req, dur_ns). Returns simulated vector finish."""
    import numpy as _np
    cum = _np.cumsum(_np.diff(bounds))
    sem = [_T0 + c * _RATE + _RCPT for c in cum]
    t = _VSTART
    for ci, dur in sorted(items):
        t = max(t, sem[ci]) + dur
    return t


def _core_items(plist, b, axis, flip, s, l, dlt, bounds):
    items = []
    for p in plist:
        if flip:
            smax = 24 - int(s[b, p, axis])
        else:
            smax = int(s[b, p, axis] + dlt[b, p, axis] + l[b, p, axis])
        vol = int(l[b, p].prod())
        items.append((_chunk_req(smax, bounds), 4 * (58 + 2 * vol) / 0.96))
    return items


def _orient_cores(s, l, dlt, assign, bounds):
    """Pick per-core chunk-major axis (+flip) and refine the proposal split
    between each batch's two cores to minimize the simulated finish."""
    orient = []
    for k in range(_NCORES):
        b, h = k // 2, k % 2
        best = None
        for axis in range(3):
            for flip in (False, True):
                if axis == 2 and flip:
                    continue  # flipped W would reverse the kept output pair
                f = _sim_finish(_core_items(assign[b][h], b, axis, flip,
                                            s, l, dlt, bounds), bounds)
                if best is None or f < best[0]:
                    best = (f, axis, flip)
        orient.append((best[1], best[2]))

    # pairwise swap refinement inside each batch
    for b in range(_B):
        for _round in range(3):
            improved = False
            a0, f0 = orient[2 * b], orient[2 * b + 1]
            A, Bp = assign[b]
            cur = max(
                _sim_finish(_core_items(A, b, a0[0], a0[1], s, l, dlt, bounds), bounds),
                _sim_finish(_core_items(Bp, b, f0[0], f0[1], s, l, dlt, bounds), bounds))
            for i in range(_PPC):
                for j in range(_PPC):
                    A2 = A.copy(); B2 = Bp.copy()
                    A2[i], B2[j] = B2[j], A2[i]
                    new = max(
                        _sim_finish(_core_items(A2, b, a0[0], a0[1], s, l, dlt, bounds), bounds),
                        _sim_finish(_core_items(B2, b, f0[0], f0[1], s, l, dlt, bounds), bounds))
                    if new < cur - 50:
                        A, Bp, cur = A2, B2, new
                        improved = True
            assign[b] = (A, Bp)
            if not improved:
                break
    return orient


def _ap_params(b, p, axis, flip, s, l, dlt):
    """Return (offset, kept_dim, reduce_dims, col_bits) for proposal p in the
    oriented layout where original axis `axis` is chunk-major (stride 576,
    optionally flipped) and the other two axes keep relative order."""
    rest = [a for a in range(3) if a != axis]
    stride_of = {axis: _SD, rest[0]: _SH, rest[1]: 1}
    sv = [int(x) for x in s[b, p]]
    lv = [int(x) for x in l[b, p]]
    dv = [int(x) for x in dlt[b, p]]
    if flip:
        sv[axis] = 24 - sv[axis] - lv[axis] - dv[axis]
    # octant loop runs over o' for D,H bits; col uses real o (= 1-o' on the
    # flipped axis). kept dim = original W axis (col stride 1).
    kept = [dv[2] * stride_of[2], 2]
    red = [[stride_of[0], lv[0]], [stride_of[1], lv[1]], [stride_of[2], lv[2]]]
    return sv, lv, dv, stride_of, kept, red


def _build_program_raw(s, l, dlt, assign, orient, n_chunks=8):
    """Raw Bacc build (no TileContext): manual semaphores, Switch dispatch.

    Avoids Tile's start/end all-engine event-semaphore butterflies and the
    sequential-If IRAM walk; each core takes one aligned jump into its own
    body and pages in exactly one IRAM block.
    """
    import concourse.bacc as bacc
    import concourse.bass as bass_mod
    import concourse.mybir as mybir
    from concourse.ap import AP

    # Bass.__init__ unconditionally memsets 4 const tiles on GpSimd and then
    # runs an all-engine event-semaphore barrier (~4us of start latency on
    # HW).  This kernel never reads const_aps, so skip both during
    # construction only.
    orig_memset = bass_mod.BassGpSimd.memset
    orig_barrier = bass_mod.Bass.all_engine_barrier
    bass_mod.BassGpSimd.memset = lambda self, ap, c: None
    bass_mod.Bass.all_engine_barrier = lambda self, **kw: None
    try:
        nc = bacc.Bacc("TRN2", target_bir_lowering=False, debug=False,
                       num_devices=_NCORES)
    finally:
        bass_mod.BassGpSimd.memset = orig_memset
        bass_mod.Bass.all_engine_barrier = orig_barrier
    x_in = nc.dram_tensor("fm", [_C, _VOL], mybir.dt.float32,
                          kind="ExternalInput")
    y_out = nc.dram_tensor("out", [_C, _PPC * 8], mybir.dt.float32,
                           kind="ExternalOutput")

    bounds = _CHUNK_BOUNDS
    n_chunks = len(bounds) - 1

    from contextlib import ExitStack
    with ExitStack() as stk:
        xt = stk.enter_context(nc.sbuf_tensor("xt", [_C, _VOL], mybir.dt.float32))
        yt = stk.enter_context(nc.sbuf_tensor("yt", [_C, _PPC * 8], mybir.dt.float32))
        # one semaphore per chunk: consecutive HWDGE DMAs may complete out of
        # order across queue rows, so a single counting sem would race
        csems = [stk.enter_context(nc.semaphore(f"dma_sem{i}"))
                 for i in range(n_chunks)]
        out_sem = stk.enter_context(nc.semaphore("out_sem"))
        v_sem = stk.enter_context(nc.semaphore("v_sem"))
        ready_sem = stk.enter_context(nc.semaphore("ready_sem"))
        block = stk.enter_context(nc.Block())

        @block.sync
        def _(sync):
            # two chunks head-start, then wait until the vector engine has
            # dispatched into its Switch body: the body's IRAM fetch shares
            # the DMA engines with these loads, and an unbounded flood can
            # queue the fetch ~10us behind (seen on HW)
            for ci in range(n_chunks):
                if ci == 2:
                    sync.wait_ge(ready_sem, 1)
                sl = slice(bounds[ci] * _SD, bounds[ci + 1] * _SD)
                sync.dma_start(out=xt[:, sl], in_=x_in[:, sl]).then_inc(csems[ci], 16)
            # result write-out: only after ALL input chunks have landed (an
            # out DMA issued mid-input steals SDMA packets and delays the
            # input-chunk semaphores), in two pieces so the bulk overlaps
            # the final reduces
            sync.wait_ge(csems[n_chunks - 1], 16)
            sync.wait_ge(v_sem, _PPC * 3)
            sync.dma_start(out=y_out[:, :_PPC * 6],
                           in_=yt[:, :_PPC * 6]).then_inc(out_sem, 16)
            sync.wait_ge(v_sem, _PPC * 4)
            sync.dma_start(out=y_out[:, _PPC * 6:],
                           in_=yt[:, _PPC * 6:]).then_inc(out_sem, 16)
            sync.wait_ge(out_sem, 32)

        pid_holder = []

        @block.vector
        def _(vector):
            pid = vector.partition_id()
            pid_holder.append(pid)
            hint = vector.switch_hint(pid, _NCORES, "disp")
            base = xt[:]
            part_dim = list(base.ap[0])
            for k in vector.Switch(pid, _NCORES, hint=hint):
                vector.engine_nop().then_inc(ready_sem, 1)
                b, h = k // 2, k % 2
                axis, flip = orient[k]
                items = _core_items(assign[b][h], b, axis, flip, s, l, dlt, bounds)
                order = sorted(range(_PPC), key=lambda i: items[i][0])
                waited = 0
                for j, idx in enumerate(order):
                    p = assign[b][h][idx]
                    ci = items[idx][0]
                    while waited <= ci:
                        vector.wait_ge(csems[waited], 16)
                        waited += 1
                    sv, lv, dv, stride_of, kept, red = _ap_params(
                        b, p, axis, flip, s, l, dlt)
                    for o0p in range(2):      # D-axis region, layout space
                        for o1p in range(2):  # H-axis region, layout space
                            # col uses real region indices; the flipped axis
                            # swaps its bit (o = 1 - o')
                            o0 = 1 - o0p if (flip and axis == 0) else o0p
                            o1 = 1 - o1p if (flip and axis == 1) else o1p
                            off = ((sv[0] + o0p * dv[0]) * stride_of[0]
                                   + (sv[1] + o1p * dv[1]) * stride_of[1]
                                   + sv[2] * stride_of[2])
                            ap = AP(base.tensor, base.offset + off,
                                    [part_dim, kept] + red)
                            col = j * 8 + o0 * 4 + o1 * 2
                            vector.tensor_reduce(
                                out=yt[:, col:col + 2], in_=ap,
                                axis=mybir.AxisListType.XYZ,
                                op=mybir.AluOpType.max).then_inc(v_sem, 1)

    # bass2jax's cache_partition_id() would otherwise add a pid register
    # load on EVERY engine (~1us each, on the measured span).  Only the DVE
    # ever consumes pid here; pre-populate all caches with the one value.
    pid_sv = pid_holder[0]
    for eng in nc.engines.values():
        if eng._cached_partition_id is None:
            eng._cached_partition_id = pid_sv
    nc._cached_partition_id_multi[tuple(mybir.ALL_ENGINES)] = pid_sv

    nc.compile()
    return nc


RAW = True


def _get_program(corners, scale):
    key = (np.asarray(corners).tobytes(), int(scale))
    if key not in _cache:
        s, l, dlt = _box_params(corners, scale)
        assign = _assign_proposals(s, l, dlt)
        if RAW:
            orient = _orient_cores(s, l, dlt, assign, _CHUNK_BOUNDS)
            nc = _build_program_raw(s, l, dlt, assign, orient)
        else:
            orient = [(0, False)] * _NCORES
            nc = _build_program(s, l, dlt, assign)
        # per-core ordered proposal lists (must match the build's issue order)
        plists = []
        for k in range(_NCORES):
            b, h = k // 2, k % 2
            if RAW:
                axis, flip = orient[k]
                items = _core_items(assign[b][h], b, axis, flip, s, l, dlt,
                                    _CHUNK_BOUNDS)
                order = sorted(range(_PPC), key=lambda i: items[i][0])
                plists.append([assign[b][h][i] for i in order])
            else:
                plists.append(sorted(assign[b][h],
                                     key=lambda p: s[b, p, 0] + dlt[b, p, 0] + l[b, p, 0]))
        _cache[key] = (nc, plists, orient)
    return _cache[key]


def _install_ntff_shim():
    """The agent image's antenv lacks axon_hooks; recreate it so
    run_bass_kernel_spmd(trace=True) can capture NTFF profiles."""
    import sys
    import types
    try:
        import antenv.axon_hooks  # noqa: F401
        return
    except ImportError:
        pass
    try:
        from trn_agent_boot.trn_boot import _ntff_profile_via_ctypes
        hook = _ntff_profile_via_ctypes("/opt/axon/libaxon_pjrt.so")
        mod = types.ModuleType("antenv.axon_hooks")
        mod._hook = hook
        mod.get_axon_ntff_profile_hook = lambda: mod._hook

        def _set(h):
            mod._hook = h

        mod.set_axon_ntff_profile_hook = _set
        sys.modules["antenv.axon_hooks"] = mod
        import antenv
        antenv.axon_hooks = mod
    except Exception:
        pass


def _run(fm, corners, scale, trace=False, trace_cores=None):
    from concourse.bass_utils import run_bass_kernel_spmd
    if trace:
        _install_ntff_shim()

    fm = np.ascontiguousarray(np.asarray(fm, dtype=np.float32))
    scale = int(scale)
    nc, plists, orient = _get_program(corners, scale)

    in_maps = []
    for k in range(_NCORES):
        b = k // 2
        axis, flip = orient[k]
        vol = fm[b]                                    # [C, D, H, W]
        if axis != 0 or flip:
            rest = [a for a in range(3) if a != axis]
            vol = np.transpose(vol, (0, 1 + axis, 1 + rest[0], 1 + rest[1]))
            if flip:
                vol = vol[:, ::-1]
        in_maps.append({"fm": np.ascontiguousarray(vol).reshape(_C, _VOL)})

    kwargs = {}
    if trace:
        kwargs.update(trace=True,
                      trace_cores=trace_cores or list(range(_NCORES)))
    res = run_bass_kernel_spmd(nc, in_maps, list(range(_NCORES)), **kwargs)

    out = np.empty((_B, _P, _C, 2, 2, 2), dtype=np.float32)
    for k in range(_NCORES):
        b = k // 2
        y = res.results[k]["out"].reshape(_C, _PPC, 2, 2, 2)
        for j, p in enumerate(plists[k]):
            out[b, p] = y[:, j]
    return out, getattr(res, "exec_time_ns", None)


def kernel(fm, corners, scale=4):
    out, _ = _run(fm, corners, scale, trace=False)
    return out

